# revision 7
# baseline (speedup 1.0000x reference)
"""HGAT retrieval-kNN kernel for Trainium2, data-parallel over batch on 8 cores.

Pipeline per batch element (reference semantics):
  pre = W @ x + b                               [128, 1024]
  pairwise = -||pre_v - pre_u||^2 per vertex    [1024, 1024]
  idx = top_k(pairwise, 32) indices             [1024, 32]
  s[v,k] = q[(32v+k) % 1024] + r[idx[v,k]],  q = a1.T pre, r = a2.T pre
  H = softmax(s, axis=batch)

Rank-equivalent distance (per row v, constants in v dropped):
  z[v,u] = x_v . (M x)_u + (c . x_u - 0.5*||pre_u||^2),
  M = W^T W [64,64], c = W^T b [64].
The column-only term rides in an augmented 65th contraction row
(lhsT row 64 = ones, rhs row 64 = c.x - 0.5*xx), so one K=65 fp32
matmul per 512-col half produces z directly in PSUM -- half the PE
work of the K=128 pre-Gram plus broadcast.

Top-32 per row: exact DVE max8/max_index rounds; the match_replace
step is rank-safe offloaded when OFFLOAD_REPLACE=True.
Host: gather r by idx, add q, softmax over batch.
"""

import numpy as np

B, C_IN, V = 32, 64, 1024
C_REL, K = 128, 32
N_CORES = 8
BPC = B // N_CORES  # 4 batches per core
NEG = -3.0e38

OFFLOAD_REPLACE = True

_cache = {}


def _build():
    import concourse.bacc as bacc
    import concourse.mybir as mybir
    import concourse.tile as tile

    dt = mybir.dt
    AF = mybir.ActivationFunctionType
    AO = mybir.AluOpType
    nc = bacc.Bacc(None, target_bir_lowering=False, debug=False)

    x_d = nc.dram_tensor("x", [BPC, C_IN, V], dt.float32, kind="ExternalInput")
    w_d = nc.dram_tensor("w", [C_REL, C_IN], dt.float32, kind="ExternalInput")
    wt_d = nc.dram_tensor("wt", [C_IN, C_REL], dt.float32, kind="ExternalInput")
    bias_d = nc.dram_tensor("bias", [C_REL, 1], dt.float32, kind="ExternalInput")
    a12_d = nc.dram_tensor("a12", [C_REL, 2], dt.float32, kind="ExternalInput")
    mi_d = nc.dram_tensor("mi", [BPC, 128, 256], dt.uint16, kind="ExternalOutput")
    qr_d = nc.dram_tensor("qr", [BPC, 2, V], dt.float32, kind="ExternalOutput")

    with tile.TileContext(nc) as tc:
        with tc.tile_pool(name="const", bufs=1) as cpool, \
             tc.tile_pool(name="perb", bufs=2) as bpool, \
             tc.tile_pool(name="zsb", bufs=3) as zpool, \
             tc.tile_pool(name="mvp", bufs=3) as mvpool, \
             tc.tile_pool(name="sgp", bufs=2) as sgpool, \
             tc.tile_pool(name="psz", bufs=2, space="PSUM") as psz, \
             tc.tile_pool(name="psp", bufs=2, space="PSUM") as psp, \
             tc.tile_pool(name="pss", bufs=2, space="PSUM") as pss:

            w_sb = cpool.tile([C_REL, C_IN], dt.float32)
            nc.sync.dma_start(w_sb[:], w_d[:])
            wt_sb = cpool.tile([C_IN, C_REL], dt.float32)
            nc.sync.dma_start(wt_sb[:], wt_d[:])
            bias_sb = cpool.tile([C_REL, 1], dt.float32)
            nc.sync.dma_start(bias_sb[:], bias_d[:])
            a12_sb = cpool.tile([C_REL, 2], dt.float32)
            nc.sync.dma_start(a12_sb[:], a12_d[:])
            ones_c = cpool.tile([C_REL, 1], dt.float32)
            nc.vector.memset(ones_c[:], 1.0)

            # m65T [64, 65]: cols 0-63 = M = W^T W, col 64 = c = W^T b.
            # (matmul computes lhsT.T @ rhs with contraction on partitions.)
            # PSUM scratch borrows a zp-pool slot to stay within 8 banks.
            m65T = cpool.tile([C_IN, 65], dt.float32)
            pm = psz.tile([128, 1024], dt.float32, tag="zp")
            nc.tensor.matmul(pm[0:C_IN, 0:64], w_sb[:], w_sb[:, 0:64],
                             start=True, stop=True)
            nc.tensor.matmul(pm[0:C_IN, 64:65], w_sb[:], bias_sb[:],
                             start=True, stop=True)
            nc.scalar.copy(m65T[:], pm[0:C_IN, 0:65])

            for b in range(BPC):
                # x lands in rows 0-63 of the augmented rhs tile; row 64
                # gets c.x - 0.5*xx once nxx and mx are ready.
                xr = bpool.tile([65, V], dt.float32, tag="xr")
                nc.sync.dma_start(xr[0:C_IN, 0:512], x_d[b][:, 0:512])
                nc.sync.dma_start(xr[0:C_IN, 512:1024], x_d[b][:, 512:1024])

                # pre = W @ x + bias; xx = sum_c pre^2; nxx = -0.5*xx
                pre_sb = bpool.tile([C_REL, V], dt.float32, tag="pre")
                pre2 = bpool.tile([C_REL, V], dt.float32, tag="pre2")
                nxx_sb = bpool.tile([1, V], dt.float32, tag="nxx")
                mxl = bpool.tile([65, V], dt.float32, tag="mxl")
                pmx = psz.tile([128, 1024], dt.float32, tag="zp")
                for h in range(2):
                    hs = slice(h * 512, (h + 1) * 512)
                    pp = psp.tile([C_REL, 512], dt.float32, tag="pp")
                    nc.tensor.matmul(pp[:], wt_sb[:], xr[0:C_IN, hs],
                                     start=True, stop=True)
                    nc.scalar.activation(pre_sb[:, hs], pp[:],
                                         AF.Identity, bias=bias_sb[:], scale=1.0)
                    nc.scalar.square(pre2[:, hs], pre_sb[:, hs])
                    pxx = pss.tile([2, 512], dt.float32, tag="pxs")
                    nc.tensor.matmul(pxx[0:1, :], ones_c[:], pre2[:, hs],
                                     start=True, stop=True)
                    nc.scalar.activation(nxx_sb[:, hs], pxx[0:1, :],
                                         AF.Copy, scale=-0.5)

                    # mx = [M; c^T] @ x : rows 0-63 -> mxl, row 64 = c.x
                    nc.tensor.matmul(pmx[0:65, hs], m65T[:], xr[0:C_IN, hs],
                                     start=True, stop=True)
                    nc.scalar.copy(mxl[0:C_IN, hs], pmx[0:C_IN, hs])
                    # xr row 64 = c.x + nxx   (PSUM + SBUF add on DVE, tiny)
                    nc.vector.tensor_tensor(out=xr[64:65, hs],
                                            in0=nxx_sb[:, hs],
                                            in1=pmx[64:65, hs],
                                            op=AO.add)
                # mxl row 64 = 1.0 (ACT: Copy(nxx*0 + 1))
                nc.scalar.activation(mxl[64:65, :], nxx_sb[:],
                                     AF.Copy, bias=1.0, scale=0.0)

                mi_sb = bpool.tile([128, 256], dt.uint16, tag="mi")
                for c in range(8):
                    # z[v,u] = x_v.(Mx)_u + (c.x_u - 0.5*xx_u), K=65 fused
                    zp = psz.tile([128, 1024], dt.float32, tag="zp")
                    for h in range(2):
                        hs = slice(h * 512, (h + 1) * 512)
                        nc.tensor.matmul(zp[:, hs],
                                         mxl[:, c * 128:(c + 1) * 128],
                                         xr[:, hs],
                                         start=True, stop=True)
                    z_sb = zpool.tile([128, V], dt.float32, tag="z")
                    nc.scalar.copy(z_sb[:], zp[:])

                    # exact top-32 (values discarded, indices kept)
                    mv_sb = mvpool.tile([128, 32], dt.float32, tag="mv")
                    if OFFLOAD_REPLACE:
                        sg_sb = sgpool.tile([128, V], dt.float32, tag="sg")
                    for rnd in range(4):
                        rs = slice(rnd * 8, (rnd + 1) * 8)
                        nc.vector.max(out=mv_sb[:, rs], in_=z_sb[:])
                        nc.vector.max_index(out=mi_sb[:, c * 32 + rnd * 8:c * 32 + rnd * 8 + 8],
                                            in_max=mv_sb[:, rs], in_values=z_sb[:])
                        if rnd < 3:
                            if OFFLOAD_REPLACE:
                                t_ap = mv_sb[:, rnd * 8 + 7:rnd * 8 + 8]
                                # mb = (z >= t) * -BIG  (DVE tensor_scalar,
                                # single-src fp32 SBUF -> 2x_2P mode)
                                nc.vector.tensor_scalar(
                                    out=sg_sb[:], in0=z_sb[:],
                                    scalar1=t_ap, scalar2=-1.0e30,
                                    op0=AO.is_ge, op1=AO.mult)
                                # z += mb  (gpsimd; -0.0 add keeps survivors exact)
                                nc.gpsimd.tensor_tensor(
                                    out=z_sb[:], in0=z_sb[:], in1=sg_sb[:],
                                    op=AO.add)
                            else:
                                nc.vector.match_replace(out=z_sb[:], in_to_replace=mv_sb[:, rs],
                                                        in_values=z_sb[:], imm_value=NEG)
                nc.sync.dma_start(mi_d[b], mi_sb[:])

                # q, r rows off the critical path
                qr_sb = bpool.tile([2, V], dt.float32, tag="qr")
                for h in range(2):
                    pqr = pss.tile([2, 512], dt.float32, tag="pxs")
                    nc.tensor.matmul(pqr[:], a12_sb[:],
                                     pre_sb[:, h * 512:(h + 1) * 512],
                                     start=True, stop=True)
                    nc.scalar.copy(qr_sb[:, h * 512:(h + 1) * 512], pqr[:])
                nc.sync.dma_start(qr_d[b], qr_sb[:])

    nc.compile()
    return nc


def _get_nc():
    if "nc" not in _cache:
        _cache["nc"] = _build()
    return _cache["nc"]


def kernel(x, W, b_conv, a):
    from concourse import bass_utils

    x = np.ascontiguousarray(np.asarray(x, dtype=np.float32))
    W = np.asarray(W, dtype=np.float32)
    b_conv = np.asarray(b_conv, dtype=np.float32)
    a = np.asarray(a, dtype=np.float32)

    nc = _get_nc()

    w = np.ascontiguousarray(W)                         # [128, 64]
    wt = np.ascontiguousarray(W.T)                      # [64, 128]
    bias = np.ascontiguousarray(b_conv[:, None])        # [128, 1]
    a12 = np.ascontiguousarray(
        np.stack([a[:C_REL, 0], a[C_REL:, 0]], axis=1)  # [128, 2]
    )
    xs = x.reshape(N_CORES, BPC, C_IN, V)

    in_maps = [{"x": np.ascontiguousarray(xs[c]), "w": w, "wt": wt,
                "bias": bias, "a12": a12}
               for c in range(N_CORES)]
    res = bass_utils.run_bass_kernel_spmd(nc, in_maps, list(range(N_CORES)))

    # host finish: gather r, add q, softmax over batch
    idx = np.empty((B, V, K), dtype=np.int64)
    q = np.empty((B, V), dtype=np.float32)
    r = np.empty((B, V), dtype=np.float32)
    for c in range(N_CORES):
        out = res.results[c]
        mi = out["mi"].reshape(BPC, 128, 8, K).transpose(0, 2, 1, 3).reshape(BPC, V, K)
        idx[c * BPC:(c + 1) * BPC] = mi
        q[c * BPC:(c + 1) * BPC] = out["qr"][:, 0, :]
        r[c * BPC:(c + 1) * BPC] = out["qr"][:, 1, :]

    pos = (np.arange(V)[:, None] * K + np.arange(K)[None, :]) % V    # [V, K]
    s = q[:, pos] + np.take_along_axis(r, idx.reshape(B, V * K), axis=1).reshape(B, V, K)
    s = s.astype(np.float32)
    m = s.max(axis=0, keepdims=True)
    e = np.exp(s - m, dtype=np.float32)
    H = e / e.sum(axis=0, keepdims=True)
    return H.astype(np.float32)
